# revision 17
# baseline (speedup 1.0000x reference)
"""Trainium2 Bass kernel for nn_DistanceLayer (shapelet min-distance).

reference semantics:
  x: (512, 1, 2048), shapelets: (128, 1, 64)
  patches = sliding windows of x (len 64, stride 1), mean-centered
  out[b, s] = min_p ||patch(b, p) - shapelets[s]||_2          -> (512, 128)

Math:
  With centered shapelets  s~ = sh - mean_l(sh):
    (w - mean(w)) . sh = w . s~
    d2[b,s,p] = A[b,p] + s2[s] - 2 w . s~
  where A = sum(w^2) - (sum w)^2/L and s2 = ||sh||^2.  min_p commutes with
  sqrt and s2 is constant over p, so PSUM only needs  A - 2 w.s~  and the
  drain is a pure min-reduce.

Layout:
  Windows p = 64j + r.  x2T[k, (b,j)] = x[b, 64j + k] for k in [0,128)
  (only 2x data), built with PE transposes.  For fixed r the window
  (b, 64j+r) is rows [r, r+64) of column (b, j); the PE needs 32-aligned
  bases, so the shift lives in zero-padded stationary weights
  Wz[k, r, s] = -2 s~[s, k - r] (host-prepared from the shapelet table,
  like the sharding hint's "replicate the small shapelet table") and each
  r is 4 bank matmuls of K=128 x N=512 over the SAME x2T tile.  The
  window term A (x-dependent, computed on device via scan ops) is folded
  with a K=32 matmul at base partition 64: an indicator-row mask
  (weights, free-broadcast) against A32 rows holding A2T[r].
  Drain: min over j per (s-partition, b), split DVE tensor_reduce vs
  ACT-copy + GpSimd min-tree, accumulated across r; then + s2, clamp,
  sqrt, transpose, store.

Data-parallel over 8 NeuronCores: 64 samples each, shapelets replicated.
"""

import os
import sys

import numpy as np

for _p in ("/root/.axon_site/_ro/trn_rl_repo", "/opt/trn_rl_repo"):
    if os.path.isdir(_p) and _p not in sys.path:
        sys.path.append(_p)

B, C, T = 512, 1, 2048
S, L = 128, 64
NCORES = 8
BPC = B // NCORES          # samples per core = 64
P = T - L + 1              # 1985 windows
J, BG, BL = 32, 4, 16      # col(b, j) = 512*(b//16) + (b%16)*32 + j
NCOL = BG * BL * J         # 2048 columns, 512 per PSUM bank

# Drain paths per r: fp32 = DVE TT-min straight off PSUM (1x);
# fp16 = ACT cast-copy PSUM->SBUF then DVE TT-min at 2x_1P.
# fp16 quantization of A-2w.s~ (|v| <~ 200) costs ~2e-4 rel on the output.
F16_RS = frozenset(r for r in range(64) if r % 3 != 2)

_STATE = {}

# dev-only experiment knobs; defaults = production behavior
_FLAGS = {"drain": True, "afold": True, "psum_bufs": 2}


def _build(nc, reps=1):
    import concourse.tile as tile
    from concourse import mybir

    f32 = mybir.dt.float32
    f32r = mybir.dt.float32r
    OP = mybir.AluOpType
    AF = mybir.ActivationFunctionType
    AX = mybir.AxisListType.X

    x_d = nc.dram_tensor("x_shard", [BPC, T], f32, kind="ExternalInput").ap()
    wz_d = nc.dram_tensor("wz", [128, 65, S], f32r, kind="ExternalInput").ap()
    s2_d = nc.dram_tensor("s2v", [S, 1], f32, kind="ExternalInput").ap()
    id_d = nc.dram_tensor("ident", [128, 128], f32, kind="ExternalInput").ap()
    mk_d = nc.dram_tensor("masks", [128, 32], f32r, kind="ExternalInput").ap()
    on_d = nc.dram_tensor("onesv", [1, 128], f32r, kind="ExternalInput").ap()
    out_d = nc.dram_tensor("out", [BPC, S], f32, kind="ExternalOutput").ap()

    with tile.TileContext(nc) as tc:
      for _it in range(reps):
        with tc.tile_pool(name=f"const{_it}", bufs=1) as constp, \
             tc.tile_pool(name=f"big{_it}", bufs=1) as bigp:

            ident = constp.tile([128, 128], f32)
            nc.scalar.dma_start(ident[:], id_d[:])
            # masks[64+t, r%32] = 1  (A-fold indicator rows, base partition 64)
            masks = constp.tile([128, 32], f32r)
            nc.scalar.dma_start(masks[:], mk_d[:])
            ones_t = constp.tile([1, 128], f32r)
            nc.scalar.dma_start(ones_t[:], on_d[:])
            s2 = constp.tile([S, 1], f32)
            nc.scalar.dma_start(s2[:], s2_d[:])
            x_sb = bigp.tile([BPC, T], f32)
            nc.sync.dma_start(x_sb[:], x_d[:])
            # Wz[k, r, s] = -2 s~[s, k - r] for r <= k < r+64 else 0; r=64 is
            # the edge window p=1984 (rows 64..127 of the j=30 column).
            Wz = bigp.tile([128, 65, S], f32r)

            x2T = bigp.tile([128, BG, BL, J], f32r)
            A2T = bigp.tile([64, BG, BL, J], f32r)
            # A32[64+t, hi, ...] = A2T[32*hi + t, ...]
            A32 = bigp.tile([128, 2, BG, BL, J], f32r)
            A2Te = constp.tile([1, BPC], f32r)
            macc_d = constp.tile([S, BPC], f32)
            # running min accumulators (two, to break the serial RAW chain)
            macc_big0 = bigp.tile([S, BG, BL, J], f32)
            nc.gpsimd.memset(macc_big0[:], 3.0e38)
            f16 = mybir.dt.float16
            macc16_0 = bigp.tile([S, BG, BL, J], f16)
            nc.gpsimd.memset(macc16_0[:], 60000.0)

            # ---- sliding-window stats:  A = sum w^2 - (sum w)^2 / L
            sq = bigp.tile([BPC, T], f32)
            nc.scalar.activation(sq[:], x_sb[:], AF.Square)
            cs = bigp.tile([BPC, T + 1], f32)
            cs2 = bigp.tile([BPC, T + 1], f32)
            nc.vector.memset(cs[:, 0:1], 0.0)
            nc.vector.memset(cs2[:, 0:1], 0.0)
            nc.vector.tensor_tensor_scan(cs[:, 1:T + 1], x_sb[:], x_sb[:],
                                         0.0, OP.add, OP.bypass)
            nc.vector.tensor_tensor_scan(cs2[:, 1:T + 1], sq[:], sq[:],
                                         0.0, OP.add, OP.bypass)
            sw = bigp.tile([BPC, P], f32)
            nc.vector.tensor_sub(sw[:], cs[:, L:T + 1], cs[:, 0:P])
            Aw = bigp.tile([BPC, P], f32)
            nc.vector.tensor_sub(Aw[:], cs2[:, L:T + 1], cs2[:, 0:P])
            # sw^2/L = Square(sw/8) since L = 64
            swsq = bigp.tile([BPC, P], f32)
            nc.scalar.activation(swsq[:], sw[:], AF.Square, scale=1.0 / 8.0)
            nc.vector.tensor_sub(Aw[:], Aw[:], swsq[:])

            with tc.tile_pool(name=f"psTr{_it}", bufs=3, space="PSUM") as psTr, \
                 tc.tile_pool(name=f"psMisc{_it}", bufs=1, space="PSUM") as psMisc:
                # x2T lower half: 32 PE transposes of 64x64 blocks of x
                for j in range(J):
                    pt = psTr.tile([64, 64], f32, tag="tr")
                    nc.tensor.transpose(pt[:], x_sb[:, 64 * j:64 * j + 64],
                                        ident[0:64, 0:64])
                    nc.scalar.mul(x2T[0:64, :, :, j],
                                  pt[:].rearrange("p (g b) -> p g b", g=BG),
                                  1.0)
                # upper half = lower half shifted by one j
                nc.sync.dma_start(x2T[64:128, :, :, 0:31], x2T[0:64, :, :, 1:32])
                nc.scalar.dma_start(Wz[:, 0:6], wz_d[:, 0:6])
                nc.gpsimd.dma_start(Wz[:, 6:22], wz_d[:, 6:22])
                nc.sync.dma_start(Wz[:, 22:43], wz_d[:, 22:43])
                nc.scalar.dma_start(Wz[:, 43:65], wz_d[:, 43:65])
                # j=31 columns never get valid data but are streamed by the
                # matmuls; zero them from known-zero regions of Wz (memset
                # can't write f32r)
                nc.gpsimd.dma_start(x2T[64:128, :, :, 31:32],
                                    Wz[64:128, 0, 0:64].rearrange(
                                        "p (g b o) -> p g b o", g=BG, b=BL))
                nc.gpsimd.dma_start(A2T[:, :, :, 31:32],
                                    Wz[0:64, 64, 0:64].rearrange(
                                        "p (g b o) -> p g b o", g=BG, b=BL))

                # A2T: same transform on A (valid j = 0..30)
                for j in range(31):
                    pt = psTr.tile([64, 64], f32, tag="tr")
                    nc.tensor.transpose(pt[:], Aw[:, 64 * j:64 * j + 64],
                                        ident[0:64, 0:64])
                    nc.scalar.mul(A2T[:, :, :, j],
                                  pt[:].rearrange("p (g b) -> p g b", g=BG),
                                  1.0)
                # A32: A2T rehomed to base partition 64 for the K=32 A-fold
                nc.sync.dma_start(A32[64:96, 0], A2T[0:32])
                nc.sync.dma_start(A32[64:96, 1], A2T[32:64])
                pe_ = psMisc.tile([1, BPC], f32, tag="e1")
                nc.tensor.transpose(pe_[:], Aw[:, 1984:1985], ident[0:64, 0:64])
                nc.scalar.mul(A2Te[:], pe_[:], 1.0)

                # edge window p = 1984: rows 64..127 of the j=30 column
                pedge = psMisc.tile([S, BPC], f32, tag="e2")
                nc.tensor.matmul(pedge[:], Wz[:, 64, :],
                                 x2T[:, :, :, 30],
                                 start=True, stop=False)
                nc.tensor.matmul(pedge[:], ones_t[:],
                                 A2Te[:], start=False, stop=True)
                nc.vector.tensor_copy(macc_d[:], pedge[:])

            # ---- main sweep over r: A - 2 w.s~ into PSUM, min-reduce out
            with tc.tile_pool(name=f"psB{_it}", bufs=_FLAGS["psum_bufs"],
                              space="PSUM") as psB, \
                 tc.tile_pool(name=f"drain{_it}", bufs=3) as drp:
                n16 = 0
                for r in range(64):
                    ps = psB.tile([S, BG, BL, J], f32, tag="ps")
                    for bg in range(BG):
                        nc.tensor.matmul(ps[:, bg], Wz[:, r, :],
                                         x2T[:, bg],
                                         start=True, stop=not _FLAGS["afold"])
                    mrow = masks[64:96, r % 32:r % 32 + 1]
                    mw = mrow.broadcast_to([32, S])
                    if _FLAGS["afold"]:
                        for bg in range(BG):
                            nc.tensor.matmul(ps[:, bg], mw,
                                             A32[64:96, r // 32, bg],
                                             start=False, stop=True)
                    if not _FLAGS["drain"]:
                        pass
                    elif r in F16_RS:
                        sb16 = drp.tile([S, BG, BL, J], f16)
                        nc.scalar.mul(sb16[:], ps[:], 1.0)
                        nc.vector.tensor_tensor(macc16_0[:], sb16[:],
                                                macc16_0[:], OP.min)
                    else:
                        nc.vector.tensor_tensor(macc_big0[:, :, :, 0:31],
                                                ps[:, :, :, 0:31],
                                                macc_big0[:, :, :, 0:31],
                                                OP.min)

                # ---- finish: fold the accumulators, + s2, clamp, sqrt
                nc.vector.tensor_tensor(macc_big0[:, :, :, 0:31],
                                        macc16_0[:, :, :, 0:31],
                                        macc_big0[:, :, :, 0:31], OP.min)
                mn_t = constp.tile([S, BPC], f32)
                nc.vector.tensor_reduce(
                    mn_t[:].rearrange("p (g b) -> p g b", g=BG),
                    macc_big0[:, :, :, 0:31], axis=AX, op=OP.min)
                nc.vector.tensor_tensor(macc_d[:], macc_d[:], mn_t[:], OP.min)
                nc.vector.tensor_scalar(macc_d[:], macc_d[:], s2[:], 0.0,
                                        OP.add, OP.max)
                res = constp.tile([S, BPC], f32)
                nc.scalar.activation(res[:], macc_d[:], AF.Sqrt)

            with tc.tile_pool(name=f"psC{_it}", bufs=1, space="PSUM") as psC:
                po = psC.tile([BPC, S], f32)
                nc.tensor.transpose(po[:], res[:], ident[:])
                outsb = constp.tile([BPC, S], f32)
                nc.scalar.mul(outsb[:], po[:], 1.0)
                nc.sync.dma_start(out_d[:], outsb[:])


def _masks_np():
    m = np.zeros((128, 32), dtype=np.float32)
    for t in range(32):
        m[64 + t, t] = 1.0
    return m


def _round_f32r(a):
    # fp32r = E8M11: round fp32 to nearest-even at the 12 dropped bits
    bits = np.ascontiguousarray(a, dtype=np.float32).view(np.uint32)
    rem = bits & np.uint32(0xFFF)
    lsb = (bits >> np.uint32(12)) & np.uint32(1)
    up = (rem > 0x800) | ((rem == 0x800) & (lsb == 1))
    out = (bits & np.uint32(0xFFFFF000)) + (up.astype(np.uint32) << np.uint32(12))
    return out.view(np.float32)


def _wz_np(sh):
    # sh: (S, L) float32 -> Wz (128, 65, S): Wz[k, r, s] = -2 s~[s, k-r]
    st = -2.0 * (sh - sh.mean(axis=1, keepdims=True))      # (S, L)
    wz = np.zeros((128, 65, S), dtype=np.float32)
    for r in range(65):
        wz[r:r + 64, r, :] = st.T
    return _round_f32r(wz)


def _get_nc():
    if "nc" not in _STATE:
        from concourse import bacc
        nc = bacc.Bacc("TRN2", target_bir_lowering=False, debug=False,
                       num_devices=NCORES)
        _build(nc)
        nc.compile()
        _STATE["nc"] = nc
    return _STATE["nc"]


def _in_maps(x, shapelets):
    x = np.ascontiguousarray(np.asarray(x, dtype=np.float32)).reshape(B, T)
    sh = np.ascontiguousarray(
        np.asarray(shapelets, dtype=np.float32)).reshape(S, L)
    wz = _wz_np(sh)
    s2v = (sh * sh).sum(axis=1, dtype=np.float32).reshape(S, 1)
    ident = np.eye(128, dtype=np.float32)
    masks = _masks_np()
    onesv = np.ones((1, 128), dtype=np.float32)
    return [{"x_shard": x[i * BPC:(i + 1) * BPC], "wz": wz, "s2v": s2v,
             "ident": ident, "masks": masks, "onesv": onesv}
            for i in range(NCORES)]


def kernel(x, shapelets):
    from concourse.bass_utils import run_bass_kernel_spmd
    nc = _get_nc()
    res = run_bass_kernel_spmd(nc, _in_maps(x, shapelets),
                               list(range(NCORES))).results
    return np.concatenate([res[i]["out"] for i in range(NCORES)], axis=0)


if __name__ == "__main__":
    rng = np.random.default_rng(0)
    x = rng.standard_normal((B, C, T)).astype(np.float32)
    sh = rng.standard_normal((S, C, L)).astype(np.float32)
    out = kernel(x, sh)
    print("out", out.shape, out.dtype, float(out.min()), float(out.max()))


# revision 19
# speedup vs baseline: 2.6311x; 2.6311x over previous
"""Trainium2 Bass kernel for nn_DistanceLayer (shapelet min-distance).

reference semantics:
  x: (512, 1, 2048), shapelets: (128, 1, 64)
  patches = sliding windows of x (len 64, stride 1), mean-centered
  out[b, s] = min_p ||patch(b, p) - shapelets[s]||_2          -> (512, 128)

Math:
  With centered shapelets  s~ = sh - mean_l(sh):
    (w - mean(w)) . sh = w . s~
    d2[b,s,p] = A[b,p] + s2[s] - 2 w . s~
  where A = sum(w^2) - (sum w)^2/L and s2 = ||sh||^2.  min_p commutes with
  sqrt and s2 is constant over p, so PSUM only needs  A - 2 w.s~  and the
  drain is a pure min-reduce.

Layout:
  Windows p = 64j + r.  x2T[k, (b,j)] = x[b, 64j + k] for k in [0,128)
  (only 2x data), built with PE transposes.  For fixed r the window
  (b, 64j+r) is rows [r, r+64) of column (b, j); the PE needs 32-aligned
  bases, so the shift lives in zero-padded stationary weights
  Wz[k, r, s] = -2 s~[s, k - r] (host-prepared from the shapelet table,
  like the sharding hint's "replicate the small shapelet table") and each
  r is 4 bank matmuls of K=128 x N=512 over the SAME x2T tile.  The
  window term A (x-dependent, computed on device via scan ops) is folded
  with a K=32 matmul at base partition 64: an indicator-row mask
  (weights, free-broadcast) against A32 rows holding A2T[r].
  Drain: min over j per (s-partition, b), split DVE tensor_reduce vs
  ACT-copy + GpSimd min-tree, accumulated across r; then + s2, clamp,
  sqrt, transpose, store.

Data-parallel over 8 NeuronCores: 64 samples each, shapelets replicated.
"""

import os
import sys

import numpy as np

for _p in ("/root/.axon_site/_ro/trn_rl_repo", "/opt/trn_rl_repo"):
    if os.path.isdir(_p) and _p not in sys.path:
        sys.path.append(_p)

B, C, T = 512, 1, 2048
S, L = 128, 64
NCORES = 8
BPC = B // NCORES          # samples per core = 64
P = T - L + 1              # 1985 windows
J, BG, BL = 32, 4, 16      # col(b, j) = 512*(b//16) + (b%16)*32 + j
NCOL = BG * BL * J         # 2048 columns, 512 per PSUM bank

# Drain paths per r: fp32 = DVE TT-min straight off PSUM (1x);
# fp16 = ACT cast-copy PSUM->SBUF then DVE TT-min at 2x_1P.
# fp16 quantization of A-2w.s~ (|v| <~ 200) costs ~2e-4 rel on the output.
F16_RS = frozenset(r for r in range(64) if r % 3 != 2)

_STATE = {}

# dev-only experiment knobs; defaults = production behavior
_FLAGS = {"drain": True, "afold": True, "mains": True, "psum_bufs": 2}


def _build(nc, reps=1):
    import concourse.tile as tile
    from concourse import mybir

    f32 = mybir.dt.float32
    f32r = mybir.dt.float32r
    OP = mybir.AluOpType
    AF = mybir.ActivationFunctionType
    AX = mybir.AxisListType.X

    x_d = nc.dram_tensor("x_shard", [BPC, T], f32, kind="ExternalInput").ap()
    wz_d = nc.dram_tensor("wz", [128, 65, S], f32r, kind="ExternalInput").ap()
    s2_d = nc.dram_tensor("s2v", [S, 1], f32, kind="ExternalInput").ap()
    id_d = nc.dram_tensor("ident", [128, 128], f32, kind="ExternalInput").ap()
    mk_d = nc.dram_tensor("masks", [128, 32], f32r, kind="ExternalInput").ap()
    on_d = nc.dram_tensor("onesv", [1, 128], f32r, kind="ExternalInput").ap()
    out_d = nc.dram_tensor("out", [BPC, S], f32, kind="ExternalOutput").ap()

    with tile.TileContext(nc) as tc:
      for _it in range(reps):
        with tc.tile_pool(name=f"const{_it}", bufs=1) as constp, \
             tc.tile_pool(name=f"big{_it}", bufs=1) as bigp:

            ident = constp.tile([128, 128], f32)
            nc.scalar.dma_start(ident[:], id_d[:])
            # masks[64+t, r%32] = 1  (A-fold indicator rows, base partition 64)
            masks = constp.tile([128, 32], f32r)
            nc.scalar.dma_start(masks[:], mk_d[:])
            ones_t = constp.tile([1, 128], f32r)
            nc.scalar.dma_start(ones_t[:], on_d[:])
            s2 = constp.tile([S, 1], f32)
            nc.scalar.dma_start(s2[:], s2_d[:])
            x_sb = bigp.tile([BPC, T], f32)
            nc.sync.dma_start(x_sb[:], x_d[:])
            # Wz[k, r, s] = -2 s~[s, k - r] for r <= k < r+64 else 0; r=64 is
            # the edge window p=1984 (rows 64..127 of the j=30 column).
            Wz = bigp.tile([128, 65, S], f32r)

            x2T = bigp.tile([128, BG, BL, J], f32r)
            A2T = bigp.tile([64, BG, BL, J], f32r)
            # A32[64+t, hi, ...] = A2T[32*hi + t, ...]; other rows stay 0 so
            # the K=128 A-fold contraction only picks up the indicator row
            A32 = bigp.tile([128, 2, BG, BL, J], f32r)
            nc.gpsimd.memset(A32[:].bitcast(f32), 0.0)
            A2Te = constp.tile([1, BPC], f32r)
            A2Te128 = constp.tile([128, BPC], f32r)
            nc.gpsimd.memset(A2Te128[:].bitcast(f32), 0.0)
            macc_d = constp.tile([S, BPC], f32)
            # running min accumulators (two, to break the serial RAW chain)
            macc_big0 = bigp.tile([S, BG, BL, J], f32)
            nc.gpsimd.memset(macc_big0[:], 3.0e38)
            f16 = mybir.dt.float16
            macc16_0 = bigp.tile([S, BG, BL, J], f16)
            nc.gpsimd.memset(macc16_0[:], 60000.0)

            # ---- sliding-window stats:  A = sum w^2 - (sum w)^2 / L
            sq = bigp.tile([BPC, T], f32)
            nc.scalar.activation(sq[:], x_sb[:], AF.Square)
            cs = bigp.tile([BPC, T + 1], f32)
            cs2 = bigp.tile([BPC, T + 1], f32)
            nc.vector.memset(cs[:, 0:1], 0.0)
            nc.vector.memset(cs2[:, 0:1], 0.0)
            nc.vector.tensor_tensor_scan(cs[:, 1:T + 1], x_sb[:], x_sb[:],
                                         0.0, OP.add, OP.bypass)
            nc.vector.tensor_tensor_scan(cs2[:, 1:T + 1], sq[:], sq[:],
                                         0.0, OP.add, OP.bypass)
            sw = bigp.tile([BPC, P], f32)
            nc.vector.tensor_sub(sw[:], cs[:, L:T + 1], cs[:, 0:P])
            Aw = bigp.tile([BPC, P], f32)
            nc.vector.tensor_sub(Aw[:], cs2[:, L:T + 1], cs2[:, 0:P])
            # sw^2/L = Square(sw/8) since L = 64
            swsq = bigp.tile([BPC, P], f32)
            nc.scalar.activation(swsq[:], sw[:], AF.Square, scale=1.0 / 8.0)
            nc.vector.tensor_sub(Aw[:], Aw[:], swsq[:])

            with tc.tile_pool(name=f"psTr{_it}", bufs=3, space="PSUM") as psTr, \
                 tc.tile_pool(name=f"psMisc{_it}", bufs=1, space="PSUM") as psMisc:
                # x2T lower half: 32 PE transposes of 64x64 blocks of x
                for j in range(J):
                    pt = psTr.tile([64, 64], f32, tag="tr")
                    nc.tensor.transpose(pt[:], x_sb[:, 64 * j:64 * j + 64],
                                        ident[0:64, 0:64])
                    nc.scalar.mul(x2T[0:64, :, :, j],
                                  pt[:].rearrange("p (g b) -> p g b", g=BG),
                                  1.0)
                # upper half = lower half shifted by one j
                nc.sync.dma_start(x2T[64:128, :, :, 0:31], x2T[0:64, :, :, 1:32])
                nc.scalar.dma_start(Wz[:, 0:6], wz_d[:, 0:6])
                nc.gpsimd.dma_start(Wz[:, 6:22], wz_d[:, 6:22])
                nc.sync.dma_start(Wz[:, 22:43], wz_d[:, 22:43])
                nc.scalar.dma_start(Wz[:, 43:65], wz_d[:, 43:65])
                # j=31 columns never get valid data but are streamed by the
                # matmuls; zero them from known-zero regions of Wz (memset
                # can't write f32r)
                nc.gpsimd.dma_start(x2T[64:128, :, :, 31:32],
                                    Wz[64:128, 0, 0:64].rearrange(
                                        "p (g b o) -> p g b o", g=BG, b=BL))
                nc.gpsimd.dma_start(A2T[:, :, :, 31:32],
                                    Wz[0:64, 64, 0:64].rearrange(
                                        "p (g b o) -> p g b o", g=BG, b=BL))

                # A2T: same transform on A (valid j = 0..30)
                for j in range(31):
                    pt = psTr.tile([64, 64], f32, tag="tr")
                    nc.tensor.transpose(pt[:], Aw[:, 64 * j:64 * j + 64],
                                        ident[0:64, 0:64])
                    nc.scalar.mul(A2T[:, :, :, j],
                                  pt[:].rearrange("p (g b) -> p g b", g=BG),
                                  1.0)
                # A32: A2T rehomed to base partition 64 for the K=32 A-fold
                nc.sync.dma_start(A32[64:96, 0], A2T[0:32])
                nc.sync.dma_start(A32[64:96, 1], A2T[32:64])
                pe_ = psMisc.tile([1, BPC], f32, tag="e1")
                nc.tensor.transpose(pe_[:], Aw[:, 1984:1985], ident[0:64, 0:64])
                nc.scalar.mul(A2Te[:], pe_[:], 1.0)
                nc.sync.dma_start(A2Te128[64:65, :], A2Te[:])

                # edge window p = 1984: rows 64..127 of the j=30 column
                pedge = psMisc.tile([S, BPC], f32, tag="e2")
                nc.tensor.matmul(pedge[:], Wz[:, 64, :],
                                 x2T[:, :, :, 30],
                                 start=True, stop=False)
                nc.tensor.matmul(pedge[:], masks[:, 0:1].broadcast_to([128, S]),
                                 A2Te128[:], start=False, stop=True)
                nc.vector.tensor_copy(macc_d[:], pedge[:])

            # ---- main sweep over r: A - 2 w.s~ into PSUM, min-reduce out
            with tc.tile_pool(name=f"psB{_it}", bufs=_FLAGS["psum_bufs"],
                              space="PSUM") as psB, \
                 tc.tile_pool(name=f"drain{_it}", bufs=3) as drp:
                n16 = 0
                for r in range(64):
                    ps = psB.tile([S, BG, BL, J], f32, tag="ps")
                    if _FLAGS["mains"]:
                        for bg in range(BG):
                            nc.tensor.matmul(ps[:, bg], Wz[:, r, :],
                                             x2T[:, bg],
                                             start=True,
                                             stop=not _FLAGS["afold"])
                    mrow = masks[:, r % 32:r % 32 + 1]
                    mw = mrow.broadcast_to([128, S])
                    if _FLAGS["afold"]:
                        for bg in range(BG):
                            nc.tensor.matmul(ps[:, bg], mw,
                                             A32[:, r // 32, bg],
                                             start=not _FLAGS["mains"],
                                             stop=True)
                    if not _FLAGS["drain"]:
                        pass
                    elif r in F16_RS:
                        sb16 = drp.tile([S, BG, BL, J], f16)
                        nc.scalar.mul(sb16[:], ps[:], 1.0)
                        nc.vector.tensor_tensor(macc16_0[:], sb16[:],
                                                macc16_0[:], OP.min)
                    else:
                        nc.vector.tensor_tensor(macc_big0[:, :, :, 0:31],
                                                ps[:, :, :, 0:31],
                                                macc_big0[:, :, :, 0:31],
                                                OP.min)

                # ---- finish: fold the accumulators, + s2, clamp, sqrt
                nc.vector.tensor_tensor(macc_big0[:, :, :, 0:31],
                                        macc16_0[:, :, :, 0:31],
                                        macc_big0[:, :, :, 0:31], OP.min)
                mn_t = constp.tile([S, BPC], f32)
                nc.vector.tensor_reduce(
                    mn_t[:].rearrange("p (g b) -> p g b", g=BG),
                    macc_big0[:, :, :, 0:31], axis=AX, op=OP.min)
                nc.vector.tensor_tensor(macc_d[:], macc_d[:], mn_t[:], OP.min)
                nc.vector.tensor_scalar(macc_d[:], macc_d[:], s2[:], 0.0,
                                        OP.add, OP.max)
                res = constp.tile([S, BPC], f32)
                nc.scalar.activation(res[:], macc_d[:], AF.Sqrt)

            with tc.tile_pool(name=f"psC{_it}", bufs=1, space="PSUM") as psC:
                po = psC.tile([BPC, S], f32)
                nc.tensor.transpose(po[:], res[:], ident[:])
                outsb = constp.tile([BPC, S], f32)
                nc.scalar.mul(outsb[:], po[:], 1.0)
                nc.sync.dma_start(out_d[:], outsb[:])


def _masks_np():
    m = np.zeros((128, 32), dtype=np.float32)
    for t in range(32):
        m[64 + t, t] = 1.0
    return m


def _round_f32r(a):
    # fp32r = E8M11: round fp32 to nearest-even at the 12 dropped bits
    bits = np.ascontiguousarray(a, dtype=np.float32).view(np.uint32)
    rem = bits & np.uint32(0xFFF)
    lsb = (bits >> np.uint32(12)) & np.uint32(1)
    up = (rem > 0x800) | ((rem == 0x800) & (lsb == 1))
    out = (bits & np.uint32(0xFFFFF000)) + (up.astype(np.uint32) << np.uint32(12))
    return out.view(np.float32)


def _wz_np(sh):
    # sh: (S, L) float32 -> Wz (128, 65, S): Wz[k, r, s] = -2 s~[s, k-r]
    st = -2.0 * (sh - sh.mean(axis=1, keepdims=True))      # (S, L)
    wz = np.zeros((128, 65, S), dtype=np.float32)
    for r in range(65):
        wz[r:r + 64, r, :] = st.T
    return _round_f32r(wz)


def _get_nc():
    if "nc" not in _STATE:
        from concourse import bacc
        nc = bacc.Bacc("TRN2", target_bir_lowering=False, debug=False,
                       num_devices=NCORES)
        _build(nc)
        nc.compile()
        _STATE["nc"] = nc
    return _STATE["nc"]


def _in_maps(x, shapelets):
    x = np.ascontiguousarray(np.asarray(x, dtype=np.float32)).reshape(B, T)
    sh = np.ascontiguousarray(
        np.asarray(shapelets, dtype=np.float32)).reshape(S, L)
    wz = _wz_np(sh)
    s2v = (sh * sh).sum(axis=1, dtype=np.float32).reshape(S, 1)
    ident = np.eye(128, dtype=np.float32)
    masks = _masks_np()
    onesv = np.ones((1, 128), dtype=np.float32)
    return [{"x_shard": x[i * BPC:(i + 1) * BPC], "wz": wz, "s2v": s2v,
             "ident": ident, "masks": masks, "onesv": onesv}
            for i in range(NCORES)]


def kernel(x, shapelets):
    from concourse.bass_utils import run_bass_kernel_spmd
    nc = _get_nc()
    res = run_bass_kernel_spmd(nc, _in_maps(x, shapelets),
                               list(range(NCORES))).results
    return np.concatenate([res[i]["out"] for i in range(NCORES)], axis=0)


if __name__ == "__main__":
    rng = np.random.default_rng(0)
    x = rng.standard_normal((B, C, T)).astype(np.float32)
    sh = rng.standard_normal((S, C, L)).astype(np.float32)
    out = kernel(x, sh)
    print("out", out.shape, out.dtype, float(out.min()), float(out.max()))
